# revision 86
# baseline (speedup 1.0000x reference)
"""Causal self-attention with RoPE, tensor-parallel over 8 NeuronCores.

Sharding: 8 cores = 2 (batch) x 4 (head-groups of 4 heads).
Each core computes q/k/v projections for its 4 heads, RoPE, causal
softmax(QK^T)V, and a partial output projection (its rows of Wo).
Host sums the 4 partials per batch and adds bo.

v3: the four big GEMMs (q/k/v projections and the Wo projection) run in
fp8-e4m3 DoubleRow mode (0.5 cycles/row, 2 k-tiles per instruction)
with full error compensation: for each k-tile pair, three DoubleRow
matmuls accumulate W8.x8 + dW8.x8 + W8.xr8 where x = x8 + xr8 and
W*32 = W8 + dW8 are exact two-level fp8 splits (residuals stored
unscaled, captured by fp8 subnormals).  3 x 0.5 = 1.5 rows/pair vs
2.0 bf16 rows, i.e. 25% fewer PE cycles at ~bf16 accuracy.  Weights are
scaled by 32 on host (e4m3 max 240); the 32x factors ride through
q/k/v, cancel in softmax via the exp scale, and are divided out of the
final 1024x-scaled Wo partials.  Scores, softmax, and A@V stay bf16.
Output partials are written bf16 and summed on host in f32.
"""

import math
import os
import sys

sys.path.insert(0, "/opt/trn_rl_repo")

import numpy as np

import concourse.bass as bass
import concourse.tile as tile
from concourse import bacc, mybir
from concourse.bass import ds, ts

B, S, D = 2, 2048, 2048
H, HD = 16, 128
ROPE_BASE = 10000.0
N_CORES = 8
N_GROUPS = 4          # head groups (tensor-parallel axis)
H_LOC = H // N_GROUPS  # heads per core
LP_MODE = os.environ.get("KBENCH_LP", "quad")   # "quad" | "ones"

QB = 512   # query block (free dim of score tiles)
KB = 128   # key block (partition dim of score tiles)
QUAD = 8   # P tiles DVE-summed per denominator ones-matmul
SB0_MODE = os.environ.get("KBENCH_SB0", "seq")
PDEPTH = int(os.environ.get("KBENCH_PDEPTH", "2"))
PS_S = int(os.environ.get("KBENCH_PSS", "2"))
PS_C = int(os.environ.get("KBENCH_PSC", "3"))
OT_ENG = os.environ.get("KBENCH_OT", "dve")
N_WARM = int(os.environ.get("KBENCH_WARM", "0"))

WSCALE = 32.0          # host-side weight scale (e4m3 max is 240)
OSCALE = 1.0 / (WSCALE * WSCALE)   # Wo partial descale
DR = mybir.MatmulPerfMode.DoubleRow


def build_core_program(Sn, Dm, h_loc, kb_plan, n_masks, lp_mode):
    """One core's program (SPMD-shared). kb_plan[qq] = [(kb, mask_idx|None)]."""
    W = h_loc * HD           # local width of Wq/Wk/Wv (columns), Wo (rows)
    KK = Dm // 128           # contraction subtiles for projections
    NSB = Sn // QB           # 512-wide s blocks
    NMB = Sn // KB           # 128-wide s blocks
    nm = QB // KB            # 128-chunks per q block
    NN = Dm // QB            # 512-wide output column blocks
    f32 = mybir.dt.float32
    f16 = mybir.dt.float16
    bf = mybir.dt.bfloat16
    f8 = mybir.dt.float8e4

    nc = bacc.Bacc("TRN2", target_bir_lowering=False, debug=False,
                   enable_asserts=True, num_devices=N_CORES)
    MASK_ENG = (nc.gpsimd.tensor_mul if os.environ.get("KBENCH_MASK") == "pool"
                else nc.vector.tensor_mul)

    x8 = nc.dram_tensor("x8", [Dm, Sn], f8, kind="ExternalInput").ap()
    xr8 = nc.dram_tensor("xr8", [Dm, Sn], f8, kind="ExternalInput").ap()
    wq = nc.dram_tensor("wq", [Dm, W], f8, kind="ExternalInput").ap()
    dwq = nc.dram_tensor("dwq", [Dm, W], f8, kind="ExternalInput").ap()
    wk = nc.dram_tensor("wk", [Dm, W], f8, kind="ExternalInput").ap()
    dwk = nc.dram_tensor("dwk", [Dm, W], f8, kind="ExternalInput").ap()
    wv = nc.dram_tensor("wv", [Dm, W], f8, kind="ExternalInput").ap()
    dwv = nc.dram_tensor("dwv", [Dm, W], f8, kind="ExternalInput").ap()
    wo = nc.dram_tensor("wo", [W, Dm], f8, kind="ExternalInput").ap()
    dwo = nc.dram_tensor("dwo", [W, Dm], f8, kind="ExternalInput").ap()
    bq = nc.dram_tensor("bq", [HD, h_loc], f32, kind="ExternalInput").ap()
    bk = nc.dram_tensor("bk", [HD, h_loc], f32, kind="ExternalInput").ap()
    bv = nc.dram_tensor("bv", [1, W], f32, kind="ExternalInput").ap()
    cos2 = nc.dram_tensor("cos2", [HD, Sn], bf, kind="ExternalInput").ap()
    sinS = nc.dram_tensor("sinS", [HD, Sn], bf, kind="ExternalInput").ap()
    prot = nc.dram_tensor("prot", [HD, HD], bf, kind="ExternalInput").ap()
    if n_masks:
        pmask = nc.dram_tensor("pmask", [n_masks, KB, QB], bf,
                               kind="ExternalInput").ap()
    out = nc.dram_tensor("out", [Sn, Dm], bf, kind="ExternalOutput").ap()

    # q,k carry a WSCALE^2 factor into the scores; fold it into the exp scale
    scale = 1.0 / (math.sqrt(HD) * WSCALE * WSCALE)

    with tile.TileContext(nc) as tc:
        with (
            tc.tile_pool(name="const", bufs=1) as cpool,
            tc.tile_pool(name="big", bufs=1) as big,
        ):
            # persistent activations (q,k bf16, 32x-scaled)
            qb_sb = big.tile([HD, h_loc, Sn], bf, tag="qb")
            kb_sb = big.tile([HD, h_loc, Sn], bf, tag="kb")
            v_sb = big.tile([KB, NMB, W], bf, tag="v")
            wo_sb = big.tile([HD, h_loc, Dm], f8, tag="wo")
            dwo_sb = big.tile([HD, h_loc, Dm], f8, tag="dwo")

            cos2_sb = cpool.tile([HD, Sn], bf, tag="cos2")
            sinS_sb = cpool.tile([HD, Sn], bf, tag="sinS")
            bq_sb = cpool.tile([HD, h_loc], f32, tag="bq")
            bk_sb = cpool.tile([HD, h_loc], f32, tag="bk")
            bv_sb = cpool.tile([1, W], f32, tag="bv")
            ones_f = cpool.tile([128, 1], f32, tag="ones_f")
            nc.gpsimd.memset(ones_f[:], 1.0)
            ones_b = cpool.tile([128, 1], bf, tag="ones_b")
            nc.vector.tensor_copy(ones_b[:], ones_f[:])
            bvb = cpool.tile([128, W], f32, tag="bvb")
            if n_masks:
                mask_sb = cpool.tile([KB, n_masks, QB], bf, tag="mask")

            with (
                tc.tile_pool(name="wa", bufs=1) as wpool,
                tc.tile_pool(name="xa", bufs=2) as xpool,
                tc.tile_pool(name="swp", bufs=4) as spool,
                tc.tile_pool(name="psa", bufs=4, space="PSUM") as psA,
                tc.tile_pool(name="psv", bufs=2, space="PSUM") as psV,
                tc.tile_pool(name="psr", bufs=2, space="PSUM") as psR,
            ):
                wq_sb = wpool.tile([128, KK, W], f8, tag="wqr")
                dwq_sb = wpool.tile([128, KK, W], f8, tag="dwqr")
                wk_sb = wpool.tile([128, KK, W], f8, tag="wkr")
                dwk_sb = wpool.tile([128, KK, W], f8, tag="dwkr")
                wv_sb = wpool.tile([128, KK, W], f8, tag="wvr")
                dwv_sb = wpool.tile([128, KK, W], f8, tag="dwvr")

                # batched DMAs: one descriptor-generation per tensor per
                # block (HWDGE gen is ~625ns each and serializes globally).
                # [Dm, N] -> [128, KK, N] views for whole-tensor loads.
                xv = x8.rearrange("(k p) s -> p k s", p=128)
                xrv = xr8.rearrange("(k p) s -> p k s", p=128)
                wqv = wq.rearrange("(k p) w -> p k w", p=128)
                dwqv = dwq.rearrange("(k p) w -> p k w", p=128)
                wkv = wk.rearrange("(k p) w -> p k w", p=128)
                dwkv = dwk.rearrange("(k p) w -> p k w", p=128)
                wvv = wv.rearrange("(k p) w -> p k w", p=128)
                dwvv = dwv.rearrange("(k p) w -> p k w", p=128)
                wov = wo.rearrange("(h p) d -> p h d", p=128)
                dwov = dwo.rearrange("(h p) d -> p h d", p=128)

                prot_sb = cpool.tile([HD, HD], bf, tag="prot")
                x0 = xpool.tile([128, KK, QB], f8, tag="x")
                xr0 = xpool.tile([128, KK, QB], f8, tag="xr")
                # sb=0 loads are chunked along kk to match the kp-major
                # emission order, so the PE can start ~3us in and stream.
                if SB0_MODE == "twopass":
                    # mains pass needs only x0/wq/wk: stream those alone,
                    # everything else lands during the mains sweep
                    for lo, hi in ((0, 2), (2, 6), (6, 11), (11, 16)):
                        nc.sync.dma_start(x0[:, lo:hi, :],
                                          xv[:, lo:hi, ts(0, QB)])
                        nc.sync.dma_start(wq_sb[:, lo:hi, :],
                                          wqv[:, lo:hi, :])
                        nc.sync.dma_start(wk_sb[:, lo:hi, :],
                                          wkv[:, lo:hi, :])
                    nc.gpsimd.dma_start(bq_sb[:], bq[:])
                    nc.gpsimd.dma_start(bk_sb[:], bk[:])
                    nc.gpsimd.dma_start(xr0[:], xrv[:, :, ts(0, QB)])
                    nc.scalar.dma_start(dwq_sb[:], dwqv[:])
                    nc.scalar.dma_start(dwk_sb[:], dwkv[:])
                    nc.scalar.dma_start(prot_sb[:], prot[:])
                    nc.scalar.dma_start(cos2_sb[:], cos2[:])
                    nc.scalar.dma_start(sinS_sb[:], sinS[:])
                    nc.scalar.dma_start(wv_sb[:], wvv[:])
                    nc.scalar.dma_start(dwv_sb[:], dwvv[:])
                    nc.scalar.dma_start(bv_sb[:], bv[:])
                else:
                    # first kp-pair chunks fan out across all three DGE
                    # queues so descriptor generation runs in parallel
                    nc.sync.dma_start(x0[:, 0:2, :], xv[:, 0:2, ts(0, QB)])
                    nc.sync.dma_start(wq_sb[:, 0:2, :], wqv[:, 0:2, :])
                    nc.sync.dma_start(wk_sb[:, 0:2, :], wkv[:, 0:2, :])
                    nc.scalar.dma_start(dwq_sb[:, 0:2, :], dwqv[:, 0:2, :])
                    nc.scalar.dma_start(dwk_sb[:, 0:2, :], dwkv[:, 0:2, :])
                    nc.sync.dma_start(x0[:, 2:6, :], xv[:, 2:6, ts(0, QB)])
                    nc.sync.dma_start(wq_sb[:, 2:6, :], wqv[:, 2:6, :])
                    nc.sync.dma_start(wk_sb[:, 2:6, :], wkv[:, 2:6, :])
                    nc.scalar.dma_start(dwq_sb[:, 2:6, :], dwqv[:, 2:6, :])
                    nc.scalar.dma_start(dwk_sb[:, 2:6, :], dwkv[:, 2:6, :])
                    nc.gpsimd.dma_start(xr0[:, 0:2, :],
                                        xrv[:, 0:2, ts(0, QB)])
                    nc.gpsimd.dma_start(xr0[:, 2:6, :],
                                        xrv[:, 2:6, ts(0, QB)])
                    nc.sync.dma_start(x0[:, 6:, :], xv[:, 6:, ts(0, QB)])
                    nc.sync.dma_start(wq_sb[:, 6:, :], wqv[:, 6:, :])
                    nc.sync.dma_start(wk_sb[:, 6:, :], wkv[:, 6:, :])
                    nc.scalar.dma_start(dwq_sb[:, 6:, :], dwqv[:, 6:, :])
                    nc.scalar.dma_start(dwk_sb[:, 6:, :], dwkv[:, 6:, :])
                    nc.gpsimd.dma_start(xr0[:, 6:, :],
                                        xrv[:, 6:, ts(0, QB)])
                    nc.gpsimd.dma_start(bq_sb[:], bq[:])
                    nc.gpsimd.dma_start(bk_sb[:], bk[:])
                    nc.scalar.dma_start(bv_sb[:], bv[:])
                    nc.scalar.dma_start(prot_sb[:], prot[:])
                    nc.scalar.dma_start(cos2_sb[:], cos2[:])
                    nc.scalar.dma_start(sinS_sb[:], sinS[:])
                    nc.scalar.dma_start(wv_sb[:], wvv[:])
                    nc.scalar.dma_start(dwv_sb[:], dwvv[:])
                nc.gpsimd.partition_broadcast(bvb[:], bv_sb[:])
                if n_masks:
                    nc.scalar.dma_start(
                        mask_sb[:], pmask.rearrange("n p q -> p n q"))
                nc.scalar.dma_start(wo_sb[:], wov[:])
                nc.scalar.dma_start(dwo_sb[:], dwov[:])

                # warm the PE p-state during the DMA preamble: tiny 1-row
                # matmuls on the resident ones tile ramp the clock so the
                # first real matmuls start at full speed
                if N_WARM:
                    warm_ps = psR.tile([1, 1], f32, tag="rr", name="warm")
                    for i in range(N_WARM):
                        nc.tensor.matmul(warm_ps[:], ones_b[:, 0:1],
                                         ones_b[:, 0:1],
                                         start=(i == 0),
                                         stop=(i == N_WARM - 1))

                x_tiles = [(x0, xr0)]

                def rope_chunk(srct, h, sb):
                    # rotate_half via PE with a +-1 permutation matrix (no
                    # cross-partition DMA): rot = prot^T @ q, then
                    # q = q*cos + rot*sin on DVE.
                    sl = ts(sb, QB)
                    rot_ps = psR.tile([HD, QB], f32, tag="rr",
                                      name="rot_ps")
                    nc.tensor.matmul(rot_ps[:], prot_sb[:],
                                     srct[:, h, sl], start=True, stop=True)
                    sw = spool.tile([HD, QB], bf, tag="sw")
                    nc.vector.tensor_mul(sw[:], rot_ps[:], sinS_sb[:, sl])
                    nc.vector.tensor_mul(srct[:, h, sl], srct[:, h, sl],
                                         cos2_sb[:, sl])
                    nc.vector.tensor_add(srct[:, h, sl], srct[:, h, sl],
                                         sw[:])

                for sb in range(NSB):
                    x_cur, xr_cur = x_tiles[sb]
                    if sb + 1 < NSB:
                        xn = xpool.tile([128, KK, QB], f8, tag="x")
                        xrn = xpool.tile([128, KK, QB], f8, tag="xr")
                        nc.sync.dma_start(xn[:], xv[:, :, ts(sb + 1, QB)])
                        nc.sync.dma_start(xrn[:], xrv[:, :, ts(sb + 1, QB)])
                        x_tiles.append((xn, xrn))

                    def emit_v_m(m):
                        # (x8+xr8)^T (wv8+dwv8) over k-tile pairs, DoubleRow:
                        # 3 half-rate pair instrs replace 2 full-rate bf16.
                        v_ps = psV.tile([KB, W], f32, tag="pv", name="v_ps")
                        n3 = (KK // 2) * 3
                        i = 0
                        for kp in range(0, KK, 2):
                            for lhs, rhs in ((x_cur, wv_sb),
                                             (x_cur, dwv_sb),
                                             (xr_cur, wv_sb)):
                                nc.tensor.matmul(
                                    v_ps[:],
                                    lhs[:, kp:kp + 2, ts(m, KB)],
                                    rhs[:, kp:kp + 2, :],
                                    start=(i == 0), stop=(i == n3 - 1),
                                    perf_mode=DR)
                                i += 1
                        nc.vector.scalar_tensor_tensor(
                            v_sb[:, sb * nm + m, :], v_ps[:], 0.0,
                            bvb[:], op0=mybir.AluOpType.add,
                            op1=mybir.AluOpType.add)

                    def emit_v():
                        for m in range(nm):
                            emit_v_m(m)

                    n3 = (KK // 2) * 3

                    def mm(ps, cnt, h, wt, xt, kp):
                        i = cnt[id(ps)]
                        nc.tensor.matmul(ps[:],
                                         wt[:, kp:kp + 2, ts(h, HD)],
                                         xt[:, kp:kp + 2, :],
                                         start=(i == 0),
                                         stop=(i == n3 - 1),
                                         perf_mode=DR)
                        cnt[id(ps)] = i + 1

                    def finish_head(h, q_t, k_t):
                        for dst, p_t, bias in ((qb_sb, q_t, bq_sb),
                                               (kb_sb, k_t, bk_sb)):
                            nc.vector.tensor_scalar_add(
                                dst[:, h, ts(sb, QB)], p_t[:],
                                bias[:, h, None])
                            rope_chunk(dst, h, sb)

                    def emit_head(h):
                        # q and k interleaved per k-pair; x8-only terms
                        # first, xr8 terms last (xr streams in later).
                        q_t = psA.tile([HD, QB], f32, tag="pa", name="q_t")
                        k_t = psA.tile([HD, QB], f32, tag="pa", name="k_t")
                        cnt = {id(q_t): 0, id(k_t): 0}
                        for kp in range(0, KK, 2):
                            mm(q_t, cnt, h, wq_sb, x_cur, kp)
                            mm(k_t, cnt, h, wk_sb, x_cur, kp)
                            mm(q_t, cnt, h, dwq_sb, x_cur, kp)
                            mm(k_t, cnt, h, dwk_sb, x_cur, kp)
                        for kp in range(0, KK, 2):
                            mm(q_t, cnt, h, wq_sb, xr_cur, kp)
                            mm(k_t, cnt, h, wk_sb, xr_cur, kp)
                        finish_head(h, q_t, k_t)

                    def emit_head_pair(h0, h1):
                        # two heads' x8/dw terms first (streams in at the
                        # sb=0 DMA rate), then both xr sweeps (xr8 lands on
                        # the parallel Pool queue meanwhile).
                        tiles = {}
                        cnt = {}
                        for h in (h0, h1):
                            q_t = psA.tile([HD, QB], f32, tag="pa",
                                           name="q_t")
                            k_t = psA.tile([HD, QB], f32, tag="pa",
                                           name="k_t")
                            tiles[h] = (q_t, k_t)
                            cnt[id(q_t)] = 0
                            cnt[id(k_t)] = 0
                        for h in (h0, h1):
                            q_t, k_t = tiles[h]
                            for kp in range(0, KK, 2):
                                mm(q_t, cnt, h, wq_sb, x_cur, kp)
                                mm(k_t, cnt, h, wk_sb, x_cur, kp)
                                mm(q_t, cnt, h, dwq_sb, x_cur, kp)
                                mm(k_t, cnt, h, dwk_sb, x_cur, kp)
                        for h in (h0, h1):
                            q_t, k_t = tiles[h]
                            for kp in range(0, KK, 2):
                                mm(q_t, cnt, h, wq_sb, xr_cur, kp)
                                mm(k_t, cnt, h, wk_sb, xr_cur, kp)
                        for h in (h0, h1):
                            finish_head(h, *tiles[h])

                    def emit_head_mains(h):
                        # main terms only (wq8.x8): needs just 3 DMA
                        # streams, so the PE can start while dw/xr land
                        q_t = psA.tile([HD, QB], f32, tag="pa", name="q_t")
                        k_t = psA.tile([HD, QB], f32, tag="pa", name="k_t")
                        for kp in range(0, KK, 2):
                            for ps, wt in ((q_t, wq_sb), (k_t, wk_sb)):
                                nc.tensor.matmul(
                                    ps[:], wt[:, kp:kp + 2, ts(h, HD)],
                                    x_cur[:, kp:kp + 2, :],
                                    start=(kp == 0), stop=(kp == KK - 2),
                                    perf_mode=DR)
                        for dst, p_t, bias in ((qb_sb, q_t, bq_sb),
                                               (kb_sb, k_t, bk_sb)):
                            nc.vector.tensor_scalar_add(
                                dst[:, h, ts(sb, QB)], p_t[:],
                                bias[:, h, None])

                    def emit_head_corr(h):
                        # dw + xr correction terms, added into the bf16
                        # mains; RoPE (linear) runs after the add
                        q_t = psA.tile([HD, QB], f32, tag="pa", name="qc_t")
                        k_t = psA.tile([HD, QB], f32, tag="pa", name="kc_t")
                        for kp in range(0, KK, 2):
                            for ps, wt, xt in ((q_t, dwq_sb, x_cur),
                                               (k_t, dwk_sb, x_cur),
                                               (q_t, wq_sb, xr_cur),
                                               (k_t, wk_sb, xr_cur)):
                                nc.tensor.matmul(
                                    ps[:], wt[:, kp:kp + 2, ts(h, HD)],
                                    xt[:, kp:kp + 2, :],
                                    start=(kp == 0 and xt is x_cur),
                                    stop=(kp == KK - 2 and xt is xr_cur),
                                    perf_mode=DR)
                        for dst, p_t in ((qb_sb, q_t), (kb_sb, k_t)):
                            nc.vector.tensor_add(dst[:, h, ts(sb, QB)],
                                                 dst[:, h, ts(sb, QB)],
                                                 p_t[:])
                            rope_chunk(dst, h, sb)

                    def emit_v_m_split(m, corr):
                        v_ps = psV.tile([KB, W], f32, tag="pv", name="v_ps")
                        pairs = (((x_cur, dwv_sb), (xr_cur, wv_sb))
                                 if corr else ((x_cur, wv_sb),))
                        n_i = len(pairs) * (KK // 2)
                        i = 0
                        for kp in range(0, KK, 2):
                            for lhs, rhs in pairs:
                                nc.tensor.matmul(
                                    v_ps[:], lhs[:, kp:kp + 2, ts(m, KB)],
                                    rhs[:, kp:kp + 2, :],
                                    start=(i == 0), stop=(i == n_i - 1),
                                    perf_mode=DR)
                                i += 1
                        if corr:
                            nc.vector.tensor_add(v_sb[:, sb * nm + m, :],
                                                 v_sb[:, sb * nm + m, :],
                                                 v_ps[:])
                        else:
                            nc.vector.scalar_tensor_tensor(
                                v_sb[:, sb * nm + m, :], v_ps[:], 0.0,
                                bvb[:], op0=mybir.AluOpType.add,
                                op1=mybir.AluOpType.add)

                    if sb == 0:
                        if SB0_MODE == "twopass":
                            for h in range(h_loc):
                                emit_head_mains(h)
                            for h in range(h_loc):
                                emit_head_corr(h)
                            for m in range(nm):
                                emit_v_m_split(m, False)
                            for m in range(nm):
                                emit_v_m_split(m, True)
                        elif SB0_MODE == "pair":
                            emit_head_pair(0, 1)
                            emit_head_pair(2, 3)
                            emit_v()
                        else:
                            for h in range(h_loc):
                                emit_head(h)
                            emit_v()
                    else:
                        for h in range(h_loc):
                            emit_head(h)
                            emit_v_m(h)

            # ------------- Phase B + C interleaved per q-block ----------
            with (
                tc.tile_pool(name="pb", bufs=QUAD + 4) as ppool,
                tc.tile_pool(name="nb", bufs=2) as npool,
                tc.tile_pool(name="rb", bufs=2) as rbpool,
                tc.tile_pool(name="a16", bufs=2) as a16pool,
                tc.tile_pool(name="ac", bufs=2) as acache,
                tc.tile_pool(name="oc", bufs=3) as opool,
                tc.tile_pool(name="pss", bufs=PS_S, space="PSUM") as psS,
                tc.tile_pool(name="pso", bufs=8 - 1 - PS_S - PS_C,
                             space="PSUM") as psO,
                tc.tile_pool(name="psl", bufs=1, space="PSUM") as psL,
                tc.tile_pool(name="psc", bufs=PS_C, space="PSUM") as psC,
            ):
                wo_queue = []
                ot_open = {}   # m-row staging: one store per NN tiles

                def emit_wo_one(cast_eng="dve"):
                    a8_prev, ar8_prev, qq_prev, m, n = wo_queue.pop(0)
                    op = psC.tile([KB, QB], f32, tag="c", name="op")
                    i = 0
                    for hp in range(0, h_loc, 2):
                        for lhs, rhs in ((a8_prev, wo_sb),
                                         (ar8_prev, wo_sb),
                                         (a8_prev, dwo_sb)):
                            nc.tensor.matmul(
                                op[:],
                                lhs[:, hp:hp + 2, ts(m, KB)],
                                rhs[:, hp:hp + 2, ts(n, QB)],
                                start=(i == 0),
                                stop=(i == (h_loc // 2) * 3 - 1),
                                perf_mode=DR)
                            i += 1
                    row = qq_prev * nm + m
                    ot = ot_open.get(row)
                    if ot is None:
                        ot = opool.tile([KB, NN, QB], bf, tag="ot")
                        ot_open[row] = ot
                    if cast_eng == "act" or OT_ENG == "act" \
                            or (OT_ENG == "alt" and n % 2):
                        nc.scalar.mul(ot[:, n, :], op[:], 1.0)
                    else:
                        nc.vector.tensor_copy(ot[:, n, :], op[:])
                    if row == Sn // KB - 1:
                        # last row: half store then single-tile stores so
                        # the final DMA carries as little as possible
                        if n == NN // 2 - 1:
                            nc.sync.dma_start(
                                out[ts(row, KB), ds(0, (NN // 2) * QB)],
                                ot[:, 0:NN // 2, :]
                                .rearrange("p n q -> p (n q)"))
                        elif n >= NN // 2:
                            nc.sync.dma_start(
                                out[ts(row, KB), ts(n, QB)], ot[:, n, :])
                            if n == NN - 1:
                                del ot_open[row]
                    elif n == NN - 1:
                        del ot_open[row]
                        nc.sync.dma_start(out[ts(row, KB), :],
                                          ot[:].rearrange("p n q -> p (n q)"))

                def emit_lp(ctx, ap, off):
                    g = ctx["gstate"]
                    nc.tensor.matmul(ctx["lp_ps"][0:1, off:], ones_b[:],
                                     ap, start=(g == 0),
                                     stop=(g == ctx["n_lp"] - 1))
                    ctx["gstate"] = g + 1

                def flush_quad(ctx):
                    qgroup = ctx["qgroup"]
                    if not qgroup:
                        return
                    if len(qgroup) == 1:
                        src = qgroup[0]
                    else:
                        acc = ppool.tile([KB, QB], bf, tag="pacc",
                                         bufs=2)
                        nc.vector.tensor_add(acc[:], qgroup[0][:],
                                             qgroup[1][:])
                        for t in qgroup[2:]:
                            nc.vector.tensor_add(acc[:], acc[:], t[:])
                        src = acc
                    emit_lp(ctx, src[:], 0)
                    qgroup.clear()

                def emit_norm(ctx):
                    # normalization for this head -> fp16, then split into
                    # fp8 act8 + residual actr8 for the DoubleRow Wo.
                    h = ctx["h"]
                    recb = rbpool.tile([128, QB], f32, tag="recb")
                    rec = npool.tile([1, QB], f32, tag="rec")
                    nc.vector.reciprocal(rec[:], ctx["lp_ps"][:])
                    nc.gpsimd.partition_broadcast(recb[:], rec[:])
                    a16 = a16pool.tile([HD, QB], f16, tag="a16")
                    nc.vector.scalar_tensor_tensor(
                        a16[:], ctx["outp"][:], 1.0, recb[:],
                        op0=mybir.AluOpType.mult,
                        op1=mybir.AluOpType.mult)
                    nc.gpsimd.tensor_copy(ctx["act8"][:, h, :], a16[:])
                    nc.vector.scalar_tensor_tensor(
                        ctx["actr8"][:, h, :], a16[:], 1.0,
                        ctx["act8"][:, h, :],
                        op0=mybir.AluOpType.mult,
                        op1=mybir.AluOpType.subtract)

                def emit_av(ctx, pt, i, kb, off):
                    h = ctx["h"]
                    last = ctx["last"]
                    nc.tensor.matmul(ctx["outp"][:, off:],
                                     v_sb[:, kb, ts(h, HD)],
                                     pt[:, off:], start=(i == 0),
                                     stop=(i == last))
                    if lp_mode == "quad":
                        if off == 0:
                            ctx["qgroup"].append(pt)
                            if len(ctx["qgroup"]) == QUAD:
                                flush_quad(ctx)
                        else:
                            # keep full-span first: drain the open quad
                            # group before any narrow write
                            flush_quad(ctx)
                            emit_lp(ctx, pt[:, off:], off)
                    else:
                        nc.tensor.matmul(ctx["lp_ps"][0:1, off:],
                                         ones_b[:], pt[:, off:],
                                         start=(i == 0), stop=(i == last))
                    if i == last:
                        if lp_mode == "quad":
                            flush_quad(ctx)
                        emit_norm(ctx)

                # pending spans head boundaries: the next head's scores
                # interleave into the previous head's A@V drain, so the
                # exp latency at head switches never exposes the PE.
                pending = []

                def pop_pending():
                    ctx, pt, i, kb, off = pending.pop(0)
                    emit_av(ctx, pt, i, kb, off)

                for qq in range(NSB):
                    plan = kb_plan[qq]
                    act8 = acache.tile([HD, h_loc, QB], f8, tag="a8")
                    actr8 = acache.tile([HD, h_loc, QB], f8, tag="ar8")
                    ntiles = max(1, len(plan) * h_loc)
                    # last block: hold a few jobs back so they can fill
                    # the final normalization-chain latency at the end
                    njobs = len(wo_queue) - (4 if qq == NSB - 1 else 0)
                    stride = max(1, ntiles // njobs) if njobs > 0 else 0
                    tcnt = 0
                    last = len(plan) - 1
                    # masked diagonal tiles only need q >= kb*KB: skip
                    # the fully-masked [0, off) columns everywhere.
                    offs = [max(0, kb * KB - qq * QB) for kb, _ in plan]
                    # replay the grouping to count lp matmuls exactly
                    n_lp, _cnt = 0, 0
                    for o in offs:
                        if o == 0:
                            _cnt += 1
                            if _cnt == QUAD:
                                n_lp, _cnt = n_lp + 1, 0
                        else:
                            n_lp += (1 if _cnt else 0) + 1
                            _cnt = 0
                    n_lp += 1 if _cnt else 0
                    for h in range(h_loc):
                        ctx = {
                            "h": h, "last": last, "n_lp": n_lp,
                            "gstate": 0, "qgroup": [],
                            "act8": act8, "actr8": actr8,
                            "outp": psO.tile([HD, QB], f32, tag="o",
                                             name="outp"),
                            "lp_ps": psL.tile([1, QB], f32, tag="l",
                                              name="lp_ps"),
                        }
                        for i, (kb, mi) in enumerate(plan):
                            off = offs[i]
                            sp = psS.tile([KB, QB], f32, tag="s", name="sp")
                            nc.tensor.matmul(sp[:, off:],
                                             kb_sb[:, h, ts(kb, KB)],
                                             qb_sb[:, h,
                                                   ds(qq * QB + off,
                                                      QB - off)],
                                             start=True, stop=True)
                            pt = ppool.tile([KB, QB], bf, tag="p")
                            nc.scalar.activation(
                                pt[:, off:], sp[:, off:],
                                mybir.ActivationFunctionType.Exp,
                                bias=0.0, scale=scale)
                            if mi is not None:
                                MASK_ENG(pt[:, off:], pt[:, off:],
                                         mask_sb[:, mi, off:])
                            pending.append((ctx, pt, i, kb, off))
                            if len(pending) > PDEPTH:
                                pop_pending()
                            tcnt += 1
                            if njobs and stride and tcnt % stride == 0 \
                                    and tcnt > 2 and wo_queue:
                                emit_wo_one()
                    # qq boundary: drain so the wo refill sees all norms
                    while pending:
                        pop_pending()
                    while wo_queue:
                        emit_wo_one(cast_eng="act")
                    wo_queue = [(act8, actr8, qq, m, n)
                                for m in range(nm) for n in range(NN)]
                while wo_queue:
                    # Act is idle once the exp stream ends; casting there
                    # keeps the psC turnaround off the busy DVE queue
                    emit_wo_one(cast_eng="act")

    nc.compile()
    return nc


# ---------------------------------------------------------------------------
# Host side
# ---------------------------------------------------------------------------

def _bf16(a):
    import ml_dtypes
    return np.ascontiguousarray(np.asarray(a).astype(ml_dtypes.bfloat16))


def _fp8_pair(a):
    """Two-level e4m3 split: a ~= hi + lo (lo unscaled, rides subnormals)."""
    import ml_dtypes
    a = np.asarray(a, np.float32)
    hi = a.astype(ml_dtypes.float8_e4m3)
    lo = (a - hi.astype(np.float32)).astype(ml_dtypes.float8_e4m3)
    return np.ascontiguousarray(hi), np.ascontiguousarray(lo)


def _rope_tables(Sn):
    inv = 1.0 / (ROPE_BASE ** (np.arange(0, HD, 2, dtype=np.float32) / HD))
    ang = np.arange(Sn, dtype=np.float32)[:, None] * inv[None, :]
    cosT = np.cos(ang).T.astype(np.float32)          # [64, S]
    sinT = np.sin(ang).T.astype(np.float32)
    cos2 = np.concatenate([cosT, cosT], 0)           # [128, S]
    sinS = np.concatenate([sinT, sinT], 0)
    # rot = P^T q = [-q2; q1]; P[j, d] = coeff of q[j] in rot[d]
    prot = np.zeros((HD, HD), np.float32)
    half = HD // 2
    for d in range(half):
        prot[d + half, d] = -1.0
        prot[d, d + half] = 1.0
    return (np.ascontiguousarray(cos2), np.ascontiguousarray(sinS),
            np.ascontiguousarray(prot))


def _classify_mask(mask, Sn):
    """-> (kb_plan, mask_tiles). kb_plan[qq] = [(kb, mask_idx|None)]."""
    nq, nk = Sn // QB, Sn // KB
    plan = []
    uniq = {}
    tiles = []
    for qq in range(nq):
        row = []
        for kb in range(nk):
            sub = mask[qq * QB:(qq + 1) * QB, kb * KB:(kb + 1) * KB]
            if sub.max() <= -200.0:
                continue                      # exp() == 0 exactly: skip
            if np.all(sub == 0.0):
                row.append((kb, None))
                continue
            t = np.ascontiguousarray(np.exp(sub.astype(np.float64))
                                     .astype(np.float32).T)  # [KB, QB]
            key = t.tobytes()
            if key not in uniq:
                uniq[key] = len(tiles)
                tiles.append(t)
            row.append((kb, uniq[key]))
        plan.append(row)
    return plan, tiles


_CACHE = {}


def _get_runner(plan_key, Sn, Dm, h_loc, kb_plan, n_masks):
    if plan_key in _CACHE:
        return _CACHE[plan_key]
    nc = build_core_program(Sn, Dm, h_loc, kb_plan, n_masks, LP_MODE)
    runner = _make_pjrt_runner(nc, N_CORES)
    _CACHE[plan_key] = runner
    return runner


def _make_pjrt_runner(nc, n_cores):
    """Persistent jitted SPMD executor (replicates bass2jax.run_bass_via_pjrt
    multi-core path, but reusable across calls for stable timing)."""
    import jax
    from jax.sharding import Mesh, PartitionSpec
    from jax.experimental.shard_map import shard_map
    from concourse.bass2jax import (_bass_exec_p, install_neuronx_cc_hook,
                                    partition_id_tensor)

    install_neuronx_cc_hook()
    pname = nc.partition_id_tensor.name if nc.partition_id_tensor else None
    in_names, out_names, out_avals, zero_outs = [], [], [], []
    for alloc in nc.m.functions[0].allocations:
        if not isinstance(alloc, mybir.MemoryLocationSet):
            continue
        name = alloc.memorylocations[0].name
        if alloc.kind == "ExternalInput":
            if name != pname:
                in_names.append(name)
        elif alloc.kind == "ExternalOutput":
            shape = tuple(alloc.tensor_shape)
            dtype = mybir.dt.np(alloc.dtype)
            out_names.append(name)
            out_avals.append(jax.core.ShapedArray(shape, dtype))
            zero_outs.append(np.zeros(shape, dtype))
    n_params = len(in_names)
    all_names = in_names + out_names
    if pname is not None:
        all_names = all_names + [pname]

    def _body(*args):
        operands = list(args)
        if pname is not None:
            operands.append(partition_id_tensor())
        outs = _bass_exec_p.bind(
            *operands, out_avals=tuple(out_avals), in_names=tuple(all_names),
            out_names=tuple(out_names), lowering_input_output_aliases=(),
            sim_require_finite=True, sim_require_nnan=True, nc=nc)
        return tuple(outs)

    devices = jax.devices()[:n_cores]
    mesh = Mesh(np.asarray(devices), ("core",))
    nin = n_params + len(out_names)
    jfn = jax.jit(shard_map(_body, mesh=mesh,
                            in_specs=(PartitionSpec("core"),) * nin,
                            out_specs=(PartitionSpec("core"),) * len(out_names),
                            check_rep=False),
                  keep_unused=True)

    def run(in_maps):
        concat = [np.concatenate([np.asarray(m[nm]) for m in in_maps], axis=0)
                  for nm in in_names]
        zeros = [np.zeros((n_cores * z.shape[0], *z.shape[1:]), z.dtype)
                 for z in zero_outs]
        outs = jfn(*concat, *zeros)
        return [{nm: np.asarray(outs[i]).reshape(n_cores, *out_avals[i].shape)[c]
                 for i, nm in enumerate(out_names)} for c in range(n_cores)]

    run.jfn = jfn
    run.in_names = in_names
    run.out_names = out_names
    run.zero_outs = zero_outs
    run.nc = nc
    return run


def _prep_in_maps(x, attn_mask, Wq, bq, Wk, bk, Wv, bv, Wo, mask_tiles):
    cos2, sinS, prot = _rope_tables(S)
    Wg = H_LOC * HD
    pm = (np.stack(mask_tiles, 0) if mask_tiles else None)
    cos2b, sinSb, protb = _bf16(cos2), _bf16(sinS), _bf16(prot)
    pmb = _bf16(pm) if pm is not None else None
    x = np.asarray(x, np.float32)
    xp = [_fp8_pair(x[b].T) for b in range(B)]
    in_maps = []
    for c in range(N_CORES):
        b, g = divmod(c, N_GROUPS)
        cs = slice(g * Wg, (g + 1) * Wg)
        wq8, dwq8 = _fp8_pair(WSCALE * np.asarray(Wq, np.float32)[:, cs])
        wk8, dwk8 = _fp8_pair(WSCALE * np.asarray(Wk, np.float32)[:, cs])
        wv8, dwv8 = _fp8_pair(WSCALE * np.asarray(Wv, np.float32)[:, cs])
        wo8, dwo8 = _fp8_pair(WSCALE * np.asarray(Wo, np.float32)[cs, :])
        m = {
            "x8": xp[b][0],
            "xr8": xp[b][1],
            "wq": wq8, "dwq": dwq8,
            "wk": wk8, "dwk": dwk8,
            "wv": wv8, "dwv": dwv8,
            "wo": wo8, "dwo": dwo8,
            "bq": np.ascontiguousarray(
                WSCALE * np.asarray(bq, np.float32)[cs].reshape(H_LOC, HD).T),
            "bk": np.ascontiguousarray(
                WSCALE * np.asarray(bk, np.float32)[cs].reshape(H_LOC, HD).T),
            "bv": np.ascontiguousarray(
                WSCALE * np.asarray(bv, np.float32)[cs][None, :]),
            "cos2": cos2b,
            "sinS": sinSb,
            "prot": protb,
        }
        if pmb is not None:
            m["pmask"] = pmb
        in_maps.append(m)
    return in_maps


def kernel(x, attn_mask, Wq, bq, Wk, bk, Wv, bv, Wo, bo):
    x = np.asarray(x, dtype=np.float32)
    mask = np.asarray(attn_mask, dtype=np.float32).reshape(S, S)
    kb_plan, mask_tiles = _classify_mask(mask, S)
    plan_key = (tuple(tuple(r) for r in kb_plan), len(mask_tiles), LP_MODE)
    runner = _get_runner(plan_key, S, D, H_LOC, kb_plan, len(mask_tiles))
    in_maps = _prep_in_maps(x, mask, np.asarray(Wq), np.asarray(bq),
                            np.asarray(Wk), np.asarray(bk), np.asarray(Wv),
                            np.asarray(bv), np.asarray(Wo), mask_tiles)
    results = runner(in_maps)
    out = np.empty((B, S, D), np.float32)
    for b in range(B):
        acc = results[b * N_GROUPS]["out"].astype(np.float32)
        for g in range(1, N_GROUPS):
            acc += results[b * N_GROUPS + g]["out"].astype(np.float32)
        # device partials carry the WSCALE^2 factor; divide it out here
        out[b] = acc * OSCALE + np.asarray(bo, np.float32)[None, :]
    return out
